# revision 42
# baseline (speedup 1.0000x reference)
"""Trainium2 Bass kernel for the SE-attention block.

Math (per batch b):
    s[n]   = sum_c x[b,c,n]
    att[c] = sum_n x[b,c,n] * s[n]
    h      = relu(bn(W1 @ att))          (BN folded into scale/bias on host)
    a      = sigmoid(W2 @ h)
    out    = x[b] * a[:, None]

Sharding: data-parallel over batch B=16 across 8 cores (2 batches/core),
weights replicated, no collectives. HBM-traffic bound: ~16.8 MB in +
16.8 MB out per core at ~380 GB/s => ~88 us floor.

v2 schedule (vs the v1 tree-sum baseline):
  - s = colsum broadcast to 128 partitions computed ENTIRELY on the PE:
    per 512-col chunk, 4 accumulating ones[128,128] @ x_t matmuls in
    fp32r (single pass, 1 cycle/row at free>=256; exact-1.0 weights so
    the only rounding is TF32-ish on x: s err ~1e-5 relative, far
    inside the gate tolerance). This removes all DVE/GpSimd tree adds
    (~51 us of vector-engine time in v1) so DVE (4 fused att passes per
    quarter, 4.7 us) keeps pace with the 5.5 us/quarter load stream.
  - W1 rank-1 accumulation and W2 gate matmuls also fp32r (half the
    instructions of LOW_HIGH fp32; h/z err ~1e-5 relative).
  - att path: DVE scalar_tensor_tensor rowsum(x * sB) stays exact fp32.
  - consts load on the ACT HWDGE ring so the SP ring's first item is an
    x tile; ACT activation tables are pre-warmed with dummy relu/sigmoid.
  - out = x * a split DVE 6 / ACT 6 / GpSimd 4 tiles per batch, each
    engine's stores issued on its own ring (SP HWDGE / ACT HWDGE /
    SWDGE) in that engine's completion order - no head-of-line blocks.
"""

import numpy as np

try:
    import concourse.bass as bass
except ImportError:  # fresh grading dir: repo not on sys.path
    import sys

    for p in ("/opt/trn_rl_repo", "/root/.axon_site/_ro/trn_rl_repo"):
        if p not in sys.path:
            sys.path.insert(0, p)
    import concourse.bass as bass

import concourse.tile as tile
from concourse import bacc, mybir
from concourse.bass_utils import run_bass_kernel_spmd

F32 = mybir.dt.float32
F32R = mybir.dt.float32r
AF = mybir.ActivationFunctionType
ALU = mybir.AluOpType

B, C, N = 16, 512, 4096
CR = 128          # squeeze dim C//4
NCORES = 8
BPC = B // NCORES  # batches per core
P = 128
CT = C // P        # channel tiles per batch
NQ = N // 4        # 1024-wide pipeline quarters
QS = 4             # quarters per batch
NCHUNK = 512       # matmul free-dim max (one psum bank)
BN_EPS = 1e-5

_nc_cache = None


def _build():
    nc = bacc.Bacc(None, target_bir_lowering=False)
    # x is declared float32r (same bits as fp32, numpy float32) so the
    # PE colsum matmuls can run single-pass; exact-fp32 consumers use
    # bitcast(F32) views of the same bytes.
    x = nc.declare_dram_parameter("x", [BPC, C, N], F32R, isOutput=False)
    ones = nc.declare_dram_parameter("ones", [P, P], F32R, isOutput=False)
    w1t = nc.declare_dram_parameter("w1t", [P, CT, CR], F32, isOutput=False)
    w2t = nc.declare_dram_parameter("w2t", [CR, C], F32, isOutput=False)
    bns = nc.declare_dram_parameter("bns", [CR, 1], F32, isOutput=False)
    bnb = nc.declare_dram_parameter("bnb", [CR, 1], F32, isOutput=False)
    y = nc.declare_dram_parameter("y", [BPC, C, N], F32, isOutput=True)

    def f(ap):
        return ap.bitcast(F32)

    with tile.TileContext(nc) as tc:
        with (
            tc.tile_pool(name="consts", bufs=1) as consts,
            tc.tile_pool(name="x", bufs=2 * CT * QS) as xpool,
            tc.tile_pool(name="big", bufs=2) as big,
            tc.tile_pool(name="small", bufs=4 * CT) as small,
            tc.tile_pool(name="psum", bufs=2, space="PSUM") as psum,
            tc.tile_pool(name="out", bufs=16) as opool,
        ):
            # consts go on the ACT HWDGE ring; the SP ring must start
            # with the first x tile.
            ones128 = consts.tile([P, P], F32R)
            nc.scalar.dma_start(out=ones128, in_=ones[:])
            # w1t is pre-arranged on the host to [p, t, o] so this is one
            # contiguous 256 KB transfer (a strided load's ~512 small
            # descriptors hog the shared HWDGE descriptor generator and
            # stall the SP load ring for ~6 us).
            w1t_sb = consts.tile([P, CT, CR], F32)
            nc.scalar.dma_start(out=w1t_sb, in_=w1t[:])
            w2t_sb = consts.tile([P, C], F32)
            nc.scalar.dma_start(out=w2t_sb, in_=w2t[:])
            bns_sb = consts.tile([P, 1], F32)
            nc.scalar.dma_start(out=bns_sb, in_=bns[:])
            bnb_sb = consts.tile([P, 1], F32)
            nc.scalar.dma_start(out=bnb_sb, in_=bnb[:])

            # Pre-warm ACT tables (relu/sigmoid/copy) on a memset scratch
            # so no table load lands at a gate, and pre-clear const DMA
            # deps with tiny dummy consumers.
            actscr = consts.tile([P, 1], F32)
            nc.gpsimd.memset(actscr, 0.0)
            scratch_sb = consts.tile([P, 1], F32)
            nc.scalar.activation(scratch_sb, actscr, AF.Relu)
            nc.scalar.activation(scratch_sb, actscr, AF.Sigmoid)
            nc.scalar.mul(scratch_sb, actscr, 1.0)
            nc.scalar.copy(scratch_sb, bns_sb)
            nc.scalar.copy(scratch_sb, bnb_sb)

            # All 32 quarter-loads up front on the SP HWDGE ring in
            # (batch, quarter) order.
            xq = [[[None] * QS for _ in range(CT)] for _ in range(BPC)]
            for b in range(BPC):
                for q in range(QS):
                    for t in range(CT):
                        tile_ = xpool.tile(
                            [P, NQ], F32R, tag="x", name=f"x_{b}_{t}_{q}"
                        )
                        nc.sync.dma_start(
                            out=tile_,
                            in_=x[b, t * P : (t + 1) * P, q * NQ : (q + 1) * NQ],
                        )
                        xq[b][t][q] = tile_

            # out = x * a engine split per batch: DVE and ACT alternate
            # (8 tiles each); GpSimd stays idle in steady state - its SBUF
            # port is an exclusive lock shared with DVE 2-port ops, and a
            # loser blocks for the whole instruction (12-15 us stalls).
            MULT_ENG = ["act", "dve"] * 8
            # att partials: one [P, 1] tile per (quarter, channel tile);
            # a column-of-[P,4] accum target slows the STT ~20%.
            attq_all = [
                [
                    [
                        small.tile([P, 1], F32, tag="attq", name=f"attq_{b}_{q}_{t}")
                        for t in range(CT)
                    ]
                    for q in range(QS)
                ]
                for b in range(BPC)
            ]

            def stream_quarter(b, q):
                # sB[m, n] = colsum over all 512 channels, broadcast to
                # all 128 partitions: 4 accumulating fp32r matmuls with
                # ones[128,128] weights per 512-col chunk; then 4 fused
                # DVE reduce passes produce the att partials.
                attq = attq_all[b]
                sb = psum.tile([P, NQ], F32, tag="sb", bufs=3, name=f"sb_{b}_{q}")
                for j in range(NQ // NCHUNK):
                    cols = slice(j * NCHUNK, (j + 1) * NCHUNK)
                    for t in range(CT):
                        nc.tensor.matmul(
                            sb[:, cols],
                            ones128[:],
                            xq[b][t][q][:, cols],
                            start=(t == 0),
                            stop=(t == CT - 1),
                        )
                for t in range(CT):
                    junk = big.tile(
                        [P, NQ], F32, tag="junk", bufs=1, name=f"junk_{b}_{q}_{t}"
                    )
                    # fused: junk = (x*1.0)*sb, attq = rowsum(junk); exact fp32
                    nc.vector.scalar_tensor_tensor(
                        out=junk,
                        in0=f(xq[b][t][q][:]),
                        scalar=1.0,
                        in1=sb,
                        op0=ALU.mult,
                        op1=ALU.mult,
                        accum_out=attq[q][t],
                    )

            def adds(b):
                # att_t = sum_q attq: 12 tiny DVE adds (tensor_tensor never
                # grabs the shared SBUF port pair)
                attq = attq_all[b]
                att_t = []
                for t in range(CT):
                    s01 = small.tile(
                        [P, 1], F32, tag="attp", bufs=12, name=f"s01_{b}_{t}"
                    )
                    s23 = small.tile(
                        [P, 1], F32, tag="attp", bufs=12, name=f"s23_{b}_{t}"
                    )
                    nc.vector.tensor_add(s01, attq[0][t], attq[1][t])
                    nc.vector.tensor_add(s23, attq[2][t], attq[3][t])
                    st = small.tile(
                        [P, 1], F32, tag="attp", bufs=12, name=f"st_{b}_{t}"
                    )
                    nc.vector.tensor_add(st, s01, s23)
                    att_t.append(st)
                return att_t

            def gate(b, att_t):
                # hpsum = sum_t W1T[t] @ att_t (4 rank-1s, kept out of the
                # stream so the PE never waits on DVE mid-stream), relu(bn),
                # W2, one batched sigmoid.
                hpsum = psum.tile([P, 1], F32, tag="mlp", name=f"hpsum_{b}")
                for t in range(CT):
                    nc.tensor.matmul(
                        hpsum,
                        w1t_sb[:, t, :],
                        att_t[t][:],
                        start=(t == 0),
                        stop=(t == CT - 1),
                    )
                hb = small.tile([P, 1], F32, tag="hb", name=f"hb_{b}")
                nc.scalar.activation(hb, hpsum, AF.Relu, bias=bnb_sb, scale=bns_sb)
                apsum = psum.tile([P, CT], F32, tag="mlp", name=f"apsum_{b}")
                for t in range(CT):
                    nc.tensor.matmul(
                        apsum[:, t : t + 1],
                        w2t_sb[:, t * P : (t + 1) * P],
                        hb[:],
                        start=True,
                        stop=True,
                    )
                avec = small.tile([P, CT], F32, tag="avec", name=f"avec_{b}")
                nc.scalar.activation(avec, apsum, AF.Sigmoid)
                return avec

            def mult_store(b, i, eng, avec, ring):
                # out-of-place out = x * a[t]. Stores split across both
                # HWDGE rings (a single ring drains stores at only ~320
                # GB/s); nothing leaks ahead of the loads because the SP
                # ring FIFO queues its stores behind all 32 loads and the
                # first multiplies only complete as the loads finish.
                t, q = i // QS, i % QS
                a_t = avec[:, t : t + 1]
                xv = f(xq[b][t][q][:])
                # b1's out tiles reuse batch 0's x buffers (dead after b0's
                # multiplies) so none of b1's 16 multiplies ever waits on a
                # store draining - the tail multiplies burst right at gate1.
                if b == 0:
                    ot = opool.tile([P, NQ], F32, tag="out", name=f"o_{b}_{t}_{q}")
                else:
                    ot = xpool.tile([P, NQ], F32, tag="x", name=f"o_{b}_{t}_{q}")
                if eng == "dve":
                    nc.vector.tensor_scalar_mul(ot, xv, a_t)
                else:
                    nc.scalar.mul(ot, xv, a_t)
                ring.dma_start(
                    out=y[b, t * P : (t + 1) * P, q * NQ : (q + 1) * NQ],
                    in_=ot,
                )

            # Emission order = scheduler priority (the Tile scheduler is
            # readiness-driven; order only breaks ties among ready work).
            # Desired per-engine preference encoded by emission position:
            #   DVE: b0 STTs > adds0 > b1 STTs > adds1 > b0 mults > b1 mults
            #   ACT: gate0 > b0 mults (ACT is idle during the load phase)
            #        > gate1 > b1 mults
            # b0 leans on ACT (10/6) because ACT is otherwise idle while
            # loads stream; b1 leans on DVE (10/6) because its multiplies
            # are the tail and DVE is ~2x faster per tile.
            for q in range(QS):
                stream_quarter(0, q)
            att0 = adds(0)
            for q in range(QS):
                stream_quarter(1, q)
            avec0 = gate(0, att0)
            # b0: 12 ACT / 4 DVE multiplies; ALL b0 stores ride the SP
            # ring, whose FIFO queues them behind the 32 loads - stores
            # can never steal load bandwidth and delay batch 1's gate.
            b0_eng = ["act", "act", "dve", "act", "act", "act", "act", "act",
                      "dve", "act", "act", "act", "act", "dve", "act", "dve"]
            for i in range(16):
                if b0_eng[i] == "act":
                    mult_store(0, i, "act", avec0, nc.sync)
            att1 = adds(1)
            avec1 = gate(1, att1)
            for i in range(16):
                if b0_eng[i] == "dve":
                    mult_store(0, i, "dve", avec0, nc.sync)
            # b1: 10 DVE / 6 ACT; its stores are post-load by construction
            # so they split across both rings for full tail drain rate.
            b1_eng = ["dve", "act", "dve", "dve", "act", "dve", "act", "dve",
                      "dve", "act", "dve", "dve", "act", "dve", "act", "dve"]
            for i in range(16):
                mult_store(1, i, b1_eng[i], avec1,
                           nc.sync if b1_eng[i] == "dve" else nc.scalar)
    return nc


def _get_nc():
    global _nc_cache
    if _nc_cache is None:
        _nc_cache = _build()
        if not _nc_cache.is_finalized():
            _nc_cache.finalize()
    return _nc_cache


def _host_prep(x, W1, gamma, beta, running_mean, running_var, W2):
    x = np.asarray(x, dtype=np.float32)
    rstd = 1.0 / np.sqrt(np.asarray(running_var, np.float32) + BN_EPS)
    bns = (np.asarray(gamma, np.float32) * rstd).reshape(CR, 1)
    bnb = (
        np.asarray(beta, np.float32)
        - np.asarray(running_mean, np.float32) * bns[:, 0]
    ).reshape(CR, 1)
    # w1t pre-arranged to the SBUF layout [p, t, o]: row (t*P + p) of W1.T
    # lands at partition p, block t -> one contiguous DMA
    w1t = np.ascontiguousarray(
        np.asarray(W1, np.float32).T.reshape(CT, P, CR).transpose(1, 0, 2)
    )  # [P, CT, CR]
    w2t = np.ascontiguousarray(np.asarray(W2, np.float32).T)  # [CR, C]
    in_maps = []
    for c in range(NCORES):
        in_maps.append(
            {
                "x": np.ascontiguousarray(x[c * BPC : (c + 1) * BPC]),
                "ones": np.ones((P, P), np.float32),
                "w1t": w1t,
                "w2t": w2t,
                "bns": np.ascontiguousarray(bns, np.float32),
                "bnb": np.ascontiguousarray(bnb, np.float32),
            }
        )
    return in_maps


def _run(inputs, **spmd_kwargs):
    in_maps = _host_prep(**inputs)
    res = run_bass_kernel_spmd(
        _get_nc(), in_maps, list(range(NCORES)), **spmd_kwargs
    )
    out = np.concatenate([res.results[c]["y"] for c in range(NCORES)], axis=0)
    return out.astype(np.float32, copy=False), res


def kernel(**inputs):
    out, _ = _run(inputs)
    return out


# revision 43
# speedup vs baseline: 1.1185x; 1.1185x over previous
"""Trainium2 Bass kernel for the SE-attention block.

Math (per batch b):
    s[n]   = sum_c x[b,c,n]
    att[c] = sum_n x[b,c,n] * s[n]
    h      = relu(bn(W1 @ att))          (BN folded into scale/bias on host)
    a      = sigmoid(W2 @ h)
    out    = x[b] * a[:, None]

Sharding: data-parallel over batch B=16 across 8 cores (2 batches/core),
weights replicated, no collectives. HBM-traffic bound: ~16.8 MB in +
16.8 MB out per core at ~380 GB/s => ~88 us floor; the schedule's job
is to keep the DMA engines saturated from first load to last store.

Schedule (exact fp32 everywhere):
  - 32 quarter loads ([128,1024] per channel-tile) up front on the SP
    HWDGE ring; consts ride the ACT ring (w1t pre-arranged on host so
    it is one contiguous DMA - a strided load's ~512 descriptors hog
    the shared HWDGE descriptor generator and stall the load ring).
  - per quarter: GpSimd pre-adds tA=x0+x1, tB=x2+x3 (tensor_tensor
    never grabs the DVE/GpSimd shared SBUF port pair), PE broadcasts
    the colsum to all 128 partitions with 2 accumulating ones[128,128]
    matmuls per 512-col chunk, DVE does 4 fused scalar_tensor_tensor
    passes: attq = rowsum(x * sB).
  - gate: att_t = sum_q attq (12 tiny DVE adds), 4 W1 rank-1 matmuls
    into PSUM (kept out of the stream so the PE never waits on DVE),
    relu(bn), 4 W2 matmuls, one batched sigmoid.
  - out = x * a IN-PLACE on the x tiles (no extra buffers, no WAR
    pacing; the tail multiplies all fire the moment the gate is ready).
    b0's multiplies run on ACT only - DVE tensor_scalar can enter
    2-port mode, and whichever of {DVE 2-port op, GpSimd op} starts
    second fully blocks (12-15 us stalls), so DVE multiplies are kept
    clear of b1's GpSimd add stream. b1: 8 DVE / 8 ACT.
  - store rings: ALL b0 stores on the SP ring - its FIFO queues them
    behind the 32 loads so no store byte can steal load bandwidth and
    delay b1's gate (the end is load-finish + gate + 22 us of b1
    stores, so load completion time is everything). b1's stores are
    post-load by construction and split across both rings.
"""

import numpy as np

try:
    import concourse.bass as bass
except ImportError:  # fresh grading dir: repo not on sys.path
    import sys

    for p in ("/opt/trn_rl_repo", "/root/.axon_site/_ro/trn_rl_repo"):
        if p not in sys.path:
            sys.path.insert(0, p)
    import concourse.bass as bass

import concourse.tile as tile
from concourse import bacc, mybir
from concourse.bass_utils import run_bass_kernel_spmd

F32 = mybir.dt.float32
AF = mybir.ActivationFunctionType
ALU = mybir.AluOpType

B, C, N = 16, 512, 4096
CR = 128          # squeeze dim C//4
NCORES = 8
BPC = B // NCORES  # batches per core
P = 128
CT = C // P        # channel tiles per batch
NQ = N // 4        # 1024-wide pipeline quarters
QS = 4             # quarters per batch
NCHUNK = 512       # matmul free-dim max (one psum bank)
BN_EPS = 1e-5

_nc_cache = None


def _build():
    nc = bacc.Bacc(None, target_bir_lowering=False)
    x = nc.declare_dram_parameter("x", [BPC, C, N], F32, isOutput=False)
    w1t = nc.declare_dram_parameter("w1t", [P, CT, CR], F32, isOutput=False)
    w2t = nc.declare_dram_parameter("w2t", [CR, C], F32, isOutput=False)
    bns = nc.declare_dram_parameter("bns", [CR, 1], F32, isOutput=False)
    bnb = nc.declare_dram_parameter("bnb", [CR, 1], F32, isOutput=False)
    y = nc.declare_dram_parameter("y", [BPC, C, N], F32, isOutput=True)

    with tile.TileContext(nc) as tc:
        with (
            tc.tile_pool(name="consts", bufs=1) as consts,
            tc.tile_pool(name="x", bufs=2 * CT * QS) as xpool,
            tc.tile_pool(name="big", bufs=2) as big,
            tc.tile_pool(name="small", bufs=4 * CT) as small,
            tc.tile_pool(name="psum", bufs=2, space="PSUM") as psum,
        ):
            # consts go on the ACT HWDGE ring; the SP ring must start
            # with the first x tile.
            ones128 = consts.tile([P, P], F32)
            nc.vector.memset(ones128, 1.0)
            w1t_sb = consts.tile([P, CT, CR], F32)
            nc.scalar.dma_start(out=w1t_sb, in_=w1t[:])
            w2t_sb = consts.tile([P, C], F32)
            nc.scalar.dma_start(out=w2t_sb, in_=w2t[:])
            bns_sb = consts.tile([P, 1], F32)
            nc.scalar.dma_start(out=bns_sb, in_=bns[:])
            bnb_sb = consts.tile([P, 1], F32)
            nc.scalar.dma_start(out=bnb_sb, in_=bnb[:])

            # Pre-warm ACT tables (relu/sigmoid/copy) on a memset scratch
            # so no table load lands at a gate.
            actscr = consts.tile([P, 1], F32)
            nc.gpsimd.memset(actscr, 0.0)
            scratch_sb = consts.tile([P, 1], F32)
            nc.scalar.activation(scratch_sb, actscr, AF.Relu)
            nc.scalar.activation(scratch_sb, actscr, AF.Sigmoid)
            nc.scalar.mul(scratch_sb, actscr, 1.0)
            nc.scalar.copy(scratch_sb, bns_sb)
            nc.scalar.copy(scratch_sb, bnb_sb)

            # All 32 quarter-loads up front on the SP HWDGE ring in
            # (batch, quarter) order.
            xq = [[[None] * QS for _ in range(CT)] for _ in range(BPC)]
            for b in range(BPC):
                for q in range(QS):
                    for t in range(CT):
                        tile_ = xpool.tile(
                            [P, NQ], F32, tag="x", name=f"x_{b}_{t}_{q}"
                        )
                        nc.sync.dma_start(
                            out=tile_,
                            in_=x[b, t * P : (t + 1) * P, q * NQ : (q + 1) * NQ],
                        )
                        xq[b][t][q] = tile_

            attq_all = [
                [
                    [
                        small.tile([P, 1], F32, tag="attq", name=f"attq_{b}_{q}_{t}")
                        for t in range(CT)
                    ]
                    for q in range(QS)
                ]
                for b in range(BPC)
            ]

            def stream_quarter(b, q):
                attq = attq_all[b]
                tA = big.tile([P, NQ], F32, tag="tA", bufs=2, name=f"tA_{b}_{q}")
                tB = big.tile([P, NQ], F32, tag="tB", bufs=2, name=f"tB_{b}_{q}")
                nc.gpsimd.tensor_add(tA, xq[b][0][q], xq[b][1][q])
                nc.gpsimd.tensor_add(tB, xq[b][2][q], xq[b][3][q])
                # sB[m, n] = colsum over all 512 channels broadcast to all
                # 128 partitions: ones[128,128] @ tA + ones @ tB per chunk.
                sb = psum.tile([P, NQ], F32, tag="sb", bufs=3, name=f"sb_{b}_{q}")
                for j in range(NQ // NCHUNK):
                    cols = slice(j * NCHUNK, (j + 1) * NCHUNK)
                    nc.tensor.matmul(
                        sb[:, cols], ones128[:], tA[:, cols],
                        start=True, stop=False,
                    )
                    nc.tensor.matmul(
                        sb[:, cols], ones128[:], tB[:, cols],
                        start=False, stop=True,
                    )
                for t in range(CT):
                    junk = big.tile(
                        [P, NQ], F32, tag="junk", bufs=2, name=f"junk_{b}_{q}_{t}"
                    )
                    # fused: junk = (x*1.0)*sb, attq = rowsum(junk)
                    nc.vector.scalar_tensor_tensor(
                        out=junk,
                        in0=xq[b][t][q],
                        scalar=1.0,
                        in1=sb,
                        op0=ALU.mult,
                        op1=ALU.mult,
                        accum_out=attq[q][t],
                    )

            def adds(b):
                # att_t = sum_q attq: 12 tiny DVE adds
                attq = attq_all[b]
                att_t = []
                for t in range(CT):
                    s01 = small.tile(
                        [P, 1], F32, tag="attp", bufs=12, name=f"s01_{b}_{t}"
                    )
                    s23 = small.tile(
                        [P, 1], F32, tag="attp", bufs=12, name=f"s23_{b}_{t}"
                    )
                    nc.vector.tensor_add(s01, attq[0][t], attq[1][t])
                    nc.vector.tensor_add(s23, attq[2][t], attq[3][t])
                    st = small.tile(
                        [P, 1], F32, tag="attp", bufs=12, name=f"st_{b}_{t}"
                    )
                    nc.vector.tensor_add(st, s01, s23)
                    att_t.append(st)
                return att_t

            def gate(b, att_t):
                hpsum = psum.tile([P, 1], F32, tag="mlp", name=f"hpsum_{b}")
                for t in range(CT):
                    nc.tensor.matmul(
                        hpsum,
                        w1t_sb[:, t, :],
                        att_t[t][:],
                        start=(t == 0),
                        stop=(t == CT - 1),
                    )
                hb = small.tile([P, 1], F32, tag="hb", name=f"hb_{b}")
                nc.scalar.activation(hb, hpsum, AF.Relu, bias=bnb_sb, scale=bns_sb)
                apsum = psum.tile([P, CT], F32, tag="mlp", name=f"apsum_{b}")
                for t in range(CT):
                    nc.tensor.matmul(
                        apsum[:, t : t + 1],
                        w2t_sb[:, t * P : (t + 1) * P],
                        hb[:],
                        start=True,
                        stop=True,
                    )
                avec = small.tile([P, CT], F32, tag="avec", name=f"avec_{b}")
                nc.scalar.activation(avec, apsum, AF.Sigmoid)
                return avec

            def mult_store(b, i, eng, avec, ring):
                # in-place out = x * a[t], store straight from the x tile
                t, q = i // QS, i % QS
                a_t = avec[:, t : t + 1]
                xv = xq[b][t][q]
                if eng == "dve":
                    nc.vector.tensor_scalar_mul(xv, xv, a_t)
                else:
                    nc.scalar.mul(xv, xv, a_t)
                ring.dma_start(
                    out=y[b, t * P : (t + 1) * P, q * NQ : (q + 1) * NQ],
                    in_=xv,
                )

            for q in range(QS):
                stream_quarter(0, q)
            att0 = adds(0)
            for q in range(QS):
                stream_quarter(1, q)
            avec0 = gate(0, att0)
            # b0: all 16 multiplies on ACT (idle during the load phase;
            # alone feeds ~430 GB/s), stores on the SP ring behind the
            # loads. DVE never runs tensor_scalar while b1's GpSimd adds
            # stream (shared-port exclusive lock).
            for i in range(16):
                mult_store(0, i, "act", avec0, nc.sync)
            att1 = adds(1)
            avec1 = gate(1, att1)
            # b1: 8 DVE / 8 ACT, stores split across both rings.
            b1_eng = ["dve", "act", "dve", "act", "dve", "act", "dve", "act",
                      "dve", "act", "dve", "act", "dve", "act", "dve", "act"]
            for i in range(16):
                mult_store(1, i, b1_eng[i], avec1,
                           nc.sync if b1_eng[i] == "dve" else nc.scalar)
    return nc


def _get_nc():
    global _nc_cache
    if _nc_cache is None:
        _nc_cache = _build()
        if not _nc_cache.is_finalized():
            _nc_cache.finalize()
    return _nc_cache


def _host_prep(x, W1, gamma, beta, running_mean, running_var, W2):
    x = np.asarray(x, dtype=np.float32)
    rstd = 1.0 / np.sqrt(np.asarray(running_var, np.float32) + BN_EPS)
    bns = (np.asarray(gamma, np.float32) * rstd).reshape(CR, 1)
    bnb = (
        np.asarray(beta, np.float32)
        - np.asarray(running_mean, np.float32) * bns[:, 0]
    ).reshape(CR, 1)
    # w1t pre-arranged to the SBUF layout [p, t, o]: row (t*P + p) of W1.T
    # lands at partition p, block t -> one contiguous DMA
    w1t = np.ascontiguousarray(
        np.asarray(W1, np.float32).T.reshape(CT, P, CR).transpose(1, 0, 2)
    )  # [P, CT, CR]
    w2t = np.ascontiguousarray(np.asarray(W2, np.float32).T)  # [CR, C]
    in_maps = []
    for c in range(NCORES):
        in_maps.append(
            {
                "x": np.ascontiguousarray(x[c * BPC : (c + 1) * BPC]),
                "w1t": w1t,
                "w2t": w2t,
                "bns": np.ascontiguousarray(bns, np.float32),
                "bnb": np.ascontiguousarray(bnb, np.float32),
            }
        )
    return in_maps


def _run(inputs, **spmd_kwargs):
    in_maps = _host_prep(**inputs)
    res = run_bass_kernel_spmd(
        _get_nc(), in_maps, list(range(NCORES)), **spmd_kwargs
    )
    out = np.concatenate([res.results[c]["y"] for c in range(NCORES)], axis=0)
    return out.astype(np.float32, copy=False), res


def kernel(**inputs):
    out, _ = _run(inputs)
    return out
